# revision 1
# baseline (speedup 1.0000x reference)
"""Trainium2 Bass kernel for a Tsit5 NeuralODE (MLP vector field).

Contract: kernel(**inputs) takes the FULL inputs of reference.setup_inputs()
and returns the FULL [101, 4096, 64] trajectory. Data-parallel: the batch is
sharded 4096 -> 8 x 512 across the chip's NeuronCores; each core integrates
its rows through 100 Tsit5 steps (6 MLP evals per step), fully unrolled into
one NEFF. Per core:
  - activations kept transposed [dim, batch] in SBUF; every matmul is a
    single N=512 moving op in float32r (TF32-like: full PE rate at N>=256,
    ~1e-4 rounding) with fp32 PSUM accumulation
  - tanh + per-channel bias on ScalarE, reading PSUM directly
  - every Runge-Kutta term is one fused VectorE scalar_tensor_tensor op
    (ps3 * hA_table[i,j,t]) + acc reading the L3 PSUM directly; the k's are
    never materialized. Stage constants (h_t * c_i * b3) are pre-folded into
    per-stage accumulators off the critical path, so a stage boundary is
    just L3 matmul -> one DVE op -> L0 matmul.
  - y state carried in fp32; a float32r twin of y feeds stage-1 matmuls
Host side (numpy): shard + transpose y0, build the h_t*A_ij / h_t*c_i*b3
tables from ts (works for any step sizes), and transpose/gather the output.
"""

import numpy as np

import concourse.bass as bass
import concourse.tile as tile
from concourse import bacc, mybir
from concourse.bass_utils import run_bass_kernel_spmd

# Tsit5 tableau (must match the reference)
A21 = 0.161
A31, A32 = -0.008480655492356989, 0.335480655492357
A41, A42, A43 = 2.8971530571054935, -6.359448489975075, 4.3622954328695815
A51, A52, A53, A54 = 5.325864828439257, -11.748883564062828, 7.4955393428898365, -0.09249506636175525
A61, A62, A63, A64, A65 = 5.86145544294642, -12.92096931784711, 8.159367898576159, -0.071584973281401, -0.028269050394068383
B1, B2, B3, B4, B5, B6 = 0.09646076681806523, 0.01, 0.4798896504144996, 1.379008574103742, -3.290069515436081, 2.324710524099774

# A[i][j] = coefficient of k_j in stage i's input (stages 2..6)
ACOEF = {
    2: {1: A21},
    3: {1: A31, 2: A32},
    4: {1: A41, 2: A42, 3: A43},
    5: {1: A51, 2: A52, 3: A53, 4: A54},
    6: {1: A61, 2: A62, 3: A63, 4: A64, 5: A65},
}
BCOEF = {1: B1, 2: B2, 3: B3, 4: B4, 5: B5, 6: B6}

# pre-acc pairs (i, j) with j <= i-2, in a fixed flat order for the hA table
PAIRS = [(i, j) for j in range(1, 5) for i in range(j + 2, 7)]
PAIRQ = {p: q for q, p in enumerate(PAIRS)}

NCORES = 8
DIM, WIDTH = 64, 256
BATCH, NT = 4096, 101
NSTEP = NT - 1
SHARD = BATCH // NCORES      # 512 rows per core
CH = 2                       # batch chains per core
CN = SHARD // CH             # 256 = chain width (also min N for f32r full rate)

F32 = mybir.dt.float32
F32R = mybir.dt.float32r
MULT = mybir.AluOpType.mult
ADD = mybir.AluOpType.add
TANH = mybir.ActivationFunctionType.Tanh

_cache = {}


def _build(nsteps=NSTEP, mm_dt=F32R, act_split=False):
    nc = bacc.Bacc("TRN2", target_bir_lowering=False, debug=False, num_devices=NCORES)

    y0t_d = nc.dram_tensor("y0t", [DIM, SHARD], F32, kind="ExternalInput").ap()
    hA_d = nc.dram_tensor("hA", [DIM, 21 * nsteps], F32, kind="ExternalInput").ap()
    hc_d = nc.dram_tensor("hc", [DIM, 6 * nsteps], F32, kind="ExternalInput").ap()
    w0_d = nc.dram_tensor("W0", [DIM, WIDTH], F32, kind="ExternalInput").ap()
    w1_d = nc.dram_tensor("W1", [WIDTH, WIDTH], F32, kind="ExternalInput").ap()
    w2_d = nc.dram_tensor("W2", [WIDTH, WIDTH], F32, kind="ExternalInput").ap()
    w3_d = nc.dram_tensor("W3", [WIDTH, DIM], F32, kind="ExternalInput").ap()
    b0_d = nc.dram_tensor("b0", [WIDTH], F32, kind="ExternalInput").ap()
    b1_d = nc.dram_tensor("b1", [WIDTH], F32, kind="ExternalInput").ap()
    b2_d = nc.dram_tensor("b2", [WIDTH], F32, kind="ExternalInput").ap()
    out_d = nc.dram_tensor("ysT", [nsteps, DIM, SHARD], F32, kind="ExternalOutput").ap()

    with tile.TileContext(nc) as tc:
        with tc.tile_pool(name="const", bufs=1) as const, \
             tc.tile_pool(name="state", bufs=2) as state, \
             tc.tile_pool(name="work", bufs=3) as work, \
             tc.tile_pool(name="psum", bufs=1, space="PSUM") as psum:

            # ---- load + round weights to f32r ----
            w0s = const.tile([DIM, 2, 128], F32, tag="w0s")
            nc.sync.dma_start(w0s[:], w0_d.rearrange("k (m j) -> k m j", j=128))
            w0 = const.tile([DIM, 2, 128], mm_dt, tag="w0")
            nc.vector.tensor_copy(w0[:], w0s[:])

            w1 = const.tile([128, 2, 2, 128], mm_dt, tag="w1")
            w2 = const.tile([128, 2, 2, 128], mm_dt, tag="w2")
            for wd, wt, nm in ((w1_d, w1, "w1"), (w2_d, w2, "w2")):
                ws = const.tile([128, 2, 2, 128], F32, tag=nm + "s", name=nm + "s")
                for t in range(2):
                    nc.sync.dma_start(
                        ws[:, t],
                        wd[t * 128:(t + 1) * 128, :].rearrange("k (m j) -> k m j", j=128),
                    )
                nc.vector.tensor_copy(wt[:], ws[:])

            w3s = const.tile([128, 2, DIM], F32, tag="w3s")
            nc.sync.dma_start(w3s[:], w3_d.rearrange("(t k) d -> k t d", k=128))
            w3 = const.tile([128, 2, DIM], mm_dt, tag="w3")
            nc.vector.tensor_copy(w3[:], w3s[:])

            # ---- biases as [128, 2] (column m = Mtile m) ----
            bt = {}
            for bd, nm in ((b0_d, "b0"), (b1_d, "b1"), (b2_d, "b2")):
                tile_b = const.tile([128, 2], F32, tag=nm + "t", name=nm + "t")
                nc.sync.dma_start(tile_b[:], bd.rearrange("(m p) -> p m", p=128))
                bt[nm] = tile_b

            # ---- per-step scalar tables ----
            hA = const.tile([DIM, 21 * nsteps], F32, tag="hA")
            nc.sync.dma_start(hA[:], hA_d)
            hc = const.tile([DIM, 6 * nsteps], F32, tag="hc")
            nc.sync.dma_start(hc[:], hc_d)

            # ---- initial state ----
            y = state.tile([DIM, SHARD], F32, tag="y", name="y")
            nc.sync.dma_start(y[:], y0t_d)
            yr = state.tile([DIM, SHARD], mm_dt, tag="yr", name="yr")
            nc.vector.tensor_copy(yr[:], y[:])

            for t in range(nsteps):
                # scalar AP helpers into the host tables
                def sA(q):
                    return hA[:, q * nsteps + t: q * nsteps + t + 1]

                def sC(q):
                    return hc[:, q * nsteps + t: q * nsteps + t + 1]

                # pre-folded accumulators: acc_i = y + c_i*h*b3 (off critical path)
                acc = {}
                for i in range(2, 7):
                    a = work.tile([DIM, SHARD], F32, tag=f"acc{i}", name=f"acc{i}")
                    nc.vector.tensor_scalar(a[:], y[:], sC(i - 2), None, ADD)
                    acc[i] = a
                accy = work.tile([DIM, SHARD], F32, tag="accy", name="accy")
                nc.vector.tensor_scalar(accy[:], y[:], sC(5), None, ADD)

                z = {}
                y_next = yr_next = None

                for s in range(1, 7):
                    rhs = yr if s == 1 else z[s]

                    # ---- MLP eval, transposed activations, chain-inner MM order ----
                    ps0 = [psum.tile([128, CH, CN], F32, tag=f"ps0_{m}", name=f"ps0_{m}") for m in range(2)]
                    for m in range(2):
                        nc.tensor.matmul(ps0[m][:], w0[:, m], rhs[:],
                                         start=True, stop=True)
                    h0 = [work.tile([128, CH, CN], mm_dt, tag=f"h0_{m}", name=f"h0_{m}") for m in range(2)]
                    for m in range(2):
                        if act_split:
                            for c in range(CH):
                                nc.scalar.activation(h0[m][:, c], ps0[m][:, c], TANH,
                                                     bias=bt["b0"][:, m:m + 1])
                        else:
                            nc.scalar.activation(h0[m][:], ps0[m][:], TANH,
                                                 bias=bt["b0"][:, m:m + 1])

                    ps1 = [psum.tile([128, CH, CN], F32, tag=f"ps1_{m}", name=f"ps1_{m}") for m in range(2)]
                    for m in range(2):
                        for k in range(2):
                            nc.tensor.matmul(ps1[m][:], w1[:, k, m], h0[k][:],
                                             start=(k == 0), stop=(k == 1))
                    h1 = [work.tile([128, CH, CN], mm_dt, tag=f"h1_{m}", name=f"h1_{m}") for m in range(2)]
                    for m in range(2):
                        if act_split:
                            for c in range(CH):
                                nc.scalar.activation(h1[m][:, c], ps1[m][:, c], TANH,
                                                     bias=bt["b1"][:, m:m + 1])
                        else:
                            nc.scalar.activation(h1[m][:], ps1[m][:], TANH,
                                                 bias=bt["b1"][:, m:m + 1])

                    ps2 = [psum.tile([128, CH, CN], F32, tag=f"ps2_{m}", name=f"ps2_{m}") for m in range(2)]
                    for m in range(2):
                        for k in range(2):
                            nc.tensor.matmul(ps2[m][:], w2[:, k, m], h1[k][:],
                                             start=(k == 0), stop=(k == 1))
                    h2 = [work.tile([128, CH, CN], mm_dt, tag=f"h2_{m}", name=f"h2_{m}") for m in range(2)]
                    for m in range(2):
                        if act_split:
                            for c in range(CH):
                                nc.scalar.activation(h2[m][:, c], ps2[m][:, c], TANH,
                                                     bias=bt["b2"][:, m:m + 1])
                        else:
                            nc.scalar.activation(h2[m][:], ps2[m][:], TANH,
                                                 bias=bt["b2"][:, m:m + 1])

                    ps3 = psum.tile([DIM, CH, CN], F32, tag="ps3", name="ps3", bufs=2)
                    for k in range(2):
                        nc.tensor.matmul(ps3[:], w3[:, k], h2[k][:],
                                         start=(k == 0), stop=(k == 1))

                    # ---- fold stage-s slope (in PSUM) into everything downstream.
                    # q-index layout in hA: z-direct i=2..6 -> q=i-2 ;
                    # pre-acc (i, j): q = 5 + PAIR[(i, j)] ; ynew B_j -> q = 15 + j - 1
                    if s < 6:
                        # z_{s+1} = (ps3 * h*A_{s+1,s}) + acc_{s+1}   [critical path]
                        zt = work.tile([DIM, SHARD], mm_dt, tag=f"z{s + 1}", name=f"z{s + 1}")
                        nc.vector.scalar_tensor_tensor(
                            zt[:], ps3[:], sA(s - 1), acc[s + 1][:], MULT, ADD)
                        z[s + 1] = zt
                        # pre-accumulate into later stages (off critical path)
                        for i in range(s + 2, 7):
                            q = 5 + PAIRQ[(i, s)]
                            nc.vector.scalar_tensor_tensor(
                                acc[i][:], ps3[:], sA(q), acc[i][:], MULT, ADD)
                        # ynew term
                        nc.vector.scalar_tensor_tensor(
                            accy[:], ps3[:], sA(15 + s - 1), accy[:], MULT, ADD)
                    else:
                        # final: yr_next (f32r, critical) and y_next (f32, for DMA/state)
                        yr_next = state.tile([DIM, SHARD], mm_dt, tag="yr", name="yr")
                        y_next = state.tile([DIM, SHARD], F32, tag="y", name="y")
                        nc.vector.scalar_tensor_tensor(
                            yr_next[:], ps3[:], sA(20), accy[:], MULT, ADD)
                        nc.vector.scalar_tensor_tensor(
                            y_next[:], ps3[:], sA(20), accy[:], MULT, ADD)

                # ---- commit step ----
                nc.sync.dma_start(out_d[t], y_next[:])
                y = y_next
                yr = yr_next

    nc.compile()
    return nc


def _get_nc(nsteps=NSTEP, **variant):
    key = (nsteps, tuple(sorted(variant.items())))
    if key not in _cache:
        _cache[key] = _build(nsteps, **variant)
    return _cache[key]


def _prepare_in_maps(ts, y0, W0, b0, W1, b1, W2, b2, W3, b3, nsteps=NSTEP):
    ts = np.asarray(ts, np.float32)
    hs = (ts[1:nsteps + 1] - ts[:nsteps]).astype(np.float64)          # [nsteps]
    b3v = np.asarray(b3, np.float64)
    # hA: [64, 21*nsteps]; q = 0..4: z-direct h*A_{i,i-1} (i=2..6);
    # q = 5..14: pre-acc h*A_ij per PAIRS; q = 15..19: h*B_j (j=1..5); q=20: h*B6
    AD = {2: {1: A21}, 3: {1: A31, 2: A32}, 4: {1: A41, 2: A42, 3: A43},
          5: {1: A51, 2: A52, 3: A53, 4: A54},
          6: {1: A61, 2: A62, 3: A63, 4: A64, 5: A65}}
    cols = []
    for i in range(2, 7):
        cols.append(hs * AD[i][i - 1])
    for (i, j) in PAIRS:
        cols.append(hs * AD[i][j])
    for j in range(1, 6):
        cols.append(hs * BCOEF[j])
    cols.append(hs * B6)
    hA = np.concatenate([np.broadcast_to(c[None, :], (DIM, nsteps)) for c in cols],
                        axis=1).astype(np.float32)
    # hc: stage prefolds c_i*h*b3[d] (i=2..6) then (sum B)*h*b3[d]
    ccols = []
    for i in range(2, 7):
        ci = sum(AD[i].values())
        ccols.append(np.outer(b3v, hs * ci))
    ccols.append(np.outer(b3v, hs * sum(BCOEF.values())))
    hc = np.concatenate(ccols, axis=1).astype(np.float32)
    common = {
        "hA": np.ascontiguousarray(hA), "hc": np.ascontiguousarray(hc),
        "W0": np.ascontiguousarray(W0, np.float32),
        "W1": np.ascontiguousarray(W1, np.float32),
        "W2": np.ascontiguousarray(W2, np.float32),
        "W3": np.ascontiguousarray(W3, np.float32),
        "b0": np.ascontiguousarray(b0, np.float32),
        "b1": np.ascontiguousarray(b1, np.float32),
        "b2": np.ascontiguousarray(b2, np.float32),
    }
    in_maps = []
    for i in range(NCORES):
        shard = np.asarray(y0[i * SHARD:(i + 1) * SHARD], np.float32)
        in_maps.append({"y0t": np.ascontiguousarray(shard.T), **common})
    return in_maps


def _run(inputs, nsteps=NSTEP, trace=False):
    nc = _get_nc(nsteps)
    in_maps = _prepare_in_maps(**inputs, nsteps=nsteps)
    res = run_bass_kernel_spmd(nc, in_maps, core_ids=list(range(NCORES)), trace=trace)
    y0 = np.asarray(inputs["y0"], np.float32)
    out = np.empty((nsteps + 1, BATCH, DIM), np.float32)
    out[0] = y0
    for i in range(NCORES):
        out[1:, i * SHARD:(i + 1) * SHARD, :] = res.results[i]["ysT"].transpose(0, 2, 1)
    return out, res


def kernel(**inputs) -> np.ndarray:
    out, _ = _run(inputs)
    return out


def _bench(inputs, iters=10, nsteps=NSTEP, **variant):
    """Time repeated device executes with a persistent jit + resident inputs.

    Returns (min_seconds_per_iter, all_times). Mirrors bass2jax.run_bass_via_pjrt's
    multi-core path but without donation so buffers stay resident across calls.
    """
    import jax
    import jax.numpy as jnp
    from jax.sharding import Mesh, PartitionSpec
    from jax.experimental.shard_map import shard_map
    from concourse import bass2jax
    from concourse import mybir as _mybir
    import time

    nc = _get_nc(nsteps, **variant)
    in_maps = _prepare_in_maps(**inputs, nsteps=nsteps)
    bass2jax.install_neuronx_cc_hook()

    partition_name = nc.partition_id_tensor.name if nc.partition_id_tensor else None
    in_names, out_names, out_avals = [], [], []
    for alloc in nc.m.functions[0].allocations:
        if not isinstance(alloc, _mybir.MemoryLocationSet):
            continue
        name = alloc.memorylocations[0].name
        if alloc.kind == "ExternalInput":
            if name != partition_name:
                in_names.append(name)
        elif alloc.kind == "ExternalOutput":
            out_names.append(name)
            out_avals.append(
                jax.core.ShapedArray(tuple(alloc.tensor_shape), _mybir.dt.np(alloc.dtype))
            )
    n_params = len(in_names)
    all_names = in_names + out_names
    if partition_name is not None:
        all_names = all_names + [partition_name]

    def _body(*args):
        operands = list(args)
        if partition_name is not None:
            operands.append(bass2jax.partition_id_tensor())
        return tuple(
            bass2jax._bass_exec_p.bind(
                *operands,
                out_avals=tuple(out_avals),
                in_names=tuple(all_names),
                out_names=tuple(out_names),
                lowering_input_output_aliases=(),
                sim_require_finite=True,
                sim_require_nnan=True,
                nc=nc,
            )
        )

    devices = jax.devices()[:NCORES]
    mesh = Mesh(np.asarray(devices), ("core",))
    n_outs = len(out_names)
    sharded = jax.jit(
        shard_map(
            _body,
            mesh=mesh,
            in_specs=(PartitionSpec("core"),) * (n_params + n_outs),
            out_specs=(PartitionSpec("core"),) * n_outs,
            check_rep=False,
        ),
        keep_unused=True,
    )
    concat_in = [
        jax.device_put(
            np.concatenate([np.asarray(in_maps[c][nm]) for c in range(NCORES)], axis=0)
        )
        for nm in in_names
    ]
    concat_zeros = [
        jax.device_put(np.zeros((NCORES * a.shape[0], *a.shape[1:]), a.dtype))
        for a in out_avals
    ]
    # warmup (compile)
    r = sharded(*concat_in, *concat_zeros)
    jax.block_until_ready(r)

    def run_n(n):
        t0 = time.perf_counter()
        rs = None
        for _ in range(n):
            rs = sharded(*concat_in, *concat_zeros)
        jax.block_until_ready(rs)
        return time.perf_counter() - t0

    run_n(3)  # pipeline warm
    slopes = []
    for _ in range(max(1, iters // 3)):
        t_small = run_n(5)
        t_big = run_n(25)
        slopes.append((t_big - t_small) / 20.0)
    return min(slopes), slopes

